# revision 1
# baseline (speedup 1.0000x reference)
"""Self-contained Trainium2 Bass kernel for nn_DiffuseMLP (GCN diffusion + MLP).

Contract: kernel(**inputs) takes FULL unsharded inputs (as in reference.setup_inputs())
and returns the FULL (128, 10) float32 output. Internally: shards edges by dst-range
across the 8 NeuronCores, runs one SPMD Bass kernel via
concourse.bass_utils.run_bass_kernel_spmd, and returns core 0's replicated output.

Algorithm per core c (dst rows [c*2048,(c+1)*2048)):
  deg reduce (host-padded layout) -> dinv = 1/sqrt(deg)
  x~T slice = dinv * x^T (PE transpose + ACT scale) -> AllGather -> x~T [N,B] bf16
  per 64-node dst window: dma_gather x~T rows by src; one-hot P=(dstl==iota)*w on DVE;
  PSUM += P^T @ msgs on PE; evict * dinv_window -> h^T tiles (bf16)
  MLP: h1 partial = sum_k hT_k^T @ W1T_k; AllReduce; relu/bias via PE-transpose+ACT;
  h2, out replicated on every core.
"""
import sys, os
for _p in ("/opt/trn_rl_repo", os.path.expanduser("~/.axon_site/_ro/trn_rl_repo")):
    if os.path.isdir(_p) and _p not in sys.path:
        sys.path.insert(0, _p)

import numpy as np
import ml_dtypes
import concourse.bass as bass
import concourse.tile as tile
import concourse.mybir as mybir
from concourse.bass_utils import run_bass_kernel_spmd
from concourse.vector_clock import ScopedClock
from concourse.masks import make_identity
from concourse import library_config

BF16 = mybir.dt.bfloat16
F32 = mybir.dt.float32
I16 = mybir.dt.int16
np_bf16 = ml_dtypes.bfloat16

B, N = 128, 16384
H, COUT = 512, 10
NCORES = 8
DPC = N // NCORES
WIN = 64
WPC = DPC // WIN
G = DPC // 128


# --------------------------------------------------------------------------
# Workarounds for this walrus build: it rejects >1 sync wait per instruction.
# --------------------------------------------------------------------------

def _patched_drain_and_barrier(self, tick_clock, wait_clock):
    nc = self.nc
    drain_inst = nc.sync.drain()
    wait_clock.add_sem_waits(drain_inst.ins, ScopedClock({None: tick_clock.global_clock}))
    si = drain_inst.ins.sync_info
    waits = list(si.on_wait or []) if si else []
    if len(waits) > 1:
        bb = nc.cur_bb.bb
        insts = bb.instructions
        assert insts[-1].name == drain_inst.ins.name
        popped = insts.pop()
        si.on_wait = [waits[-1]]
        for w in waits[:-1]:
            nop = nc.sync.nop(nofuse=True)
            nsi = nop.ins.sync_info
            if nsi is None:
                nop.ins.sync_info = mybir.SyncInfo(on_wait=[w], on_update=[])
            else:
                nsi.on_wait = [w]
        bb.add_instruction(popped)
    nc.all_engine_barrier()
    assert self.sems is not None
    popped_p = nc._tile_sem_poison_stack.pop()
    assert popped_p is self._sem_poison
    nc.clear_and_free_semaphores(list(self.sems.allocated().values()))
    nc.all_engine_barrier()


tile.TileContext._drain_and_barrier = _patched_drain_and_barrier


def legalize_waits(nc, max_waits=1):
    n_split = 0
    for fn in nc.m.functions:
        for bb in fn.blocks:
            insts = list(bb.instructions)
            out = []
            for inst in insts:
                si = inst.sync_info
                waits = list(si.on_wait or []) if si else []
                if len(waits) > max_waits:
                    n_split += 1
                    for i in range(0, len(waits) - max_waits, max_waits):
                        chunk = waits[i:i + max_waits]
                        nop = mybir.InstNoOp(
                            name=f"waitnop_{inst.name}_{i}", ins=[], outs=[],
                            sync_info=mybir.SyncInfo(on_wait=chunk, on_update=[]),
                        )
                        nop.engine = inst.engine
                        nc.register_instruction(nop, overwrite=True)
                        out.append(nop)
                    si.on_wait = waits[len(waits) - max_waits:]
                out.append(inst)
            if len(out) != len(insts):
                bb.instructions.clear()
                for i in out:
                    bb.add_instruction(i)
    return n_split


# --------------------------------------------------------------------------
# Host-side sharding / layout prep
# --------------------------------------------------------------------------

def prep(x, edge_index, edge_weight):
    src = np.asarray(edge_index[0], np.int64).astype(np.int32)
    dst = np.asarray(edge_index[1], np.int64).astype(np.int32)
    w = np.asarray(edge_weight, np.float32)
    loop = np.arange(N, dtype=np.int32)
    src = np.concatenate([src, loop])
    dst = np.concatenate([dst, loop])
    w = np.concatenate([w, np.ones(N, np.float32)])
    order = np.argsort(dst, kind="stable")
    ss, ds, ws = src[order], dst[order], w[order]

    gwin = ds // WIN
    counts = np.bincount(gwin, minlength=N // WIN).reshape(NCORES, WPC)
    caps = np.maximum(counts.max(axis=0), 1)
    caps = ((caps + 127) // 128) * 128
    CW = caps // 128
    SC = int(CW.sum())
    offs = np.concatenate([[0], np.cumsum(counts.reshape(-1))])

    gidx_all = np.zeros((NCORES, 128, SC * 8), np.int16)
    dstl_all = np.full((NCORES, 128, SC), -1.0, np_bf16)
    wv_all = np.zeros((NCORES, 128, SC), np_bf16)
    capoff = np.concatenate([[0], np.cumsum(caps)])
    for c in range(NCORES):
        srcp = np.zeros(int(caps.sum()), np.int16)
        dstlp = np.full(int(caps.sum()), -1.0, np.float32)
        wvp = np.zeros(int(caps.sum()), np.float32)
        for wslot in range(WPC):
            g = c * WPC + wslot
            s0, s1 = offs[g], offs[g + 1]
            n = s1 - s0
            o = capoff[wslot]
            srcp[o:o + n] = ss[s0:s1]
            dstlp[o:o + n] = ds[s0:s1] % WIN
            wvp[o:o + n] = ws[s0:s1]
        gidx_all[c] = np.tile(srcp.reshape(SC * 8, 16).T, (8, 1))
        dstl_all[c] = dstlp.reshape(SC, 128).T.astype(np_bf16)
        wv_all[c] = wvp.reshape(SC, 128).T.astype(np_bf16)

    degc = np.bincount(ds, minlength=N).astype(np.int64)
    width = int(max(64, ((degc.max() + 31) // 32) * 32))
    starts = np.concatenate([[0], np.cumsum(np.bincount(ds, minlength=N))])[:-1]
    rank = np.arange(len(ds)) - starts[ds]
    dl = ds % DPC
    core_of = ds // DPC
    wdeg_flat = np.zeros((NCORES, DPC, width), np.float32)
    wdeg_flat[core_of, dl, rank] = ws
    wdeg_all = np.zeros((NCORES, 128, G, width), np.float32)
    for c in range(NCORES):
        wdeg_all[c] = wdeg_flat[c].reshape(G, 128, width).transpose(1, 0, 2)

    wdegf = np.ascontiguousarray(
        wdeg_flat.reshape(N // 128, 128, width).transpose(1, 0, 2))
    return dict(gidx=gidx_all, dstl=dstl_all, wv=wv_all, wdeg=wdeg_all,
                wdegf=wdegf, CW=CW, SC=SC, width=width)


def make_inmaps(inputs):
    x = np.asarray(inputs["x"], np.float32)
    pp = prep(x, np.asarray(inputs["edge_index"]), np.asarray(inputs["edge_weight"]))
    CW, width = pp["CW"], pp["width"]
    CMAX = int(CW.max())
    W1T = np.ascontiguousarray(np.asarray(inputs["W1"], np.float32).T).astype(np_bf16)
    W2T = np.ascontiguousarray(np.asarray(inputs["W2"], np.float32).T).astype(np_bf16)
    WfcT = np.ascontiguousarray(np.asarray(inputs["Wfc"], np.float32).T).astype(np_bf16)
    b1r = np.ascontiguousarray(np.asarray(inputs["b1"], np.float32).reshape(H // 128, 128).T)
    b2r = np.ascontiguousarray(np.asarray(inputs["b2"], np.float32).reshape(H // 128, 128).T)
    bfcr = np.tile(np.asarray(inputs["bfc"], np.float32)[None, :], (128, 1))
    iotat = np.ascontiguousarray(
        np.broadcast_to(np.arange(WIN, dtype=np.float32), (128, CMAX, WIN))).astype(np_bf16)
    in_maps = []
    for c in range(NCORES):
        in_maps.append({
            "xfull": x,
            "wdeg": pp["wdeg"][c],
            "wdegf": pp["wdegf"],
            "gidx": pp["gidx"][c],
            "dstl": pp["dstl"][c],
            "wv": pp["wv"][c],
            "w1t": np.ascontiguousarray(W1T[c * DPC:(c + 1) * DPC]),
            "w2t": W2T, "wfct": WfcT,
            "b1r": b1r, "b2r": b2r, "bfcr": bfcr,
            "iotat": iotat,
        })
    return in_maps, CW, width


def build(CW, width, debug=False, skip=(), GSP=8, MBUFS=3, PBUFS=3, NQ=4):
    """CW: list[int] chunks per window-slot (len WPC); width: deg pad slots."""
    SC = int(sum(CW))
    CMAX = int(max(CW))
    nc = bass.Bass(num_swdge_queues=4)

    # ---- I/O ----
    x_d = nc.declare_dram_parameter("xfull", [128, N], F32, isOutput=False)
    wdeg_d = nc.declare_dram_parameter("wdeg", [128, G, width], F32, isOutput=False)
    wdegf_d = nc.declare_dram_parameter("wdegf", [128, N // 128, width], F32, isOutput=False)
    gidx_d = nc.declare_dram_parameter("gidx", [128, SC * 8], I16, isOutput=False)
    dstl_d = nc.declare_dram_parameter("dstl", [128, SC], BF16, isOutput=False)
    wv_d = nc.declare_dram_parameter("wv", [128, SC], BF16, isOutput=False)
    w1t_d = nc.declare_dram_parameter("w1t", [DPC, H], BF16, isOutput=False)
    w2t_d = nc.declare_dram_parameter("w2t", [H, H], BF16, isOutput=False)
    wfct_d = nc.declare_dram_parameter("wfct", [H, COUT], BF16, isOutput=False)
    b1_d = nc.declare_dram_parameter("b1r", [128, H // 128], F32, isOutput=False)
    b2_d = nc.declare_dram_parameter("b2r", [128, H // 128], F32, isOutput=False)
    bfc_d = nc.declare_dram_parameter("bfcr", [128, COUT], F32, isOutput=False)
    out_d = nc.declare_dram_parameter("out", [B, COUT], F32, isOutput=True)
    if debug:
        dbg_dinv = nc.declare_dram_parameter("dbg_dinv", [128, G], F32, isOutput=True)
        dbg_xt = nc.declare_dram_parameter("dbg_xt", [N, B], BF16, isOutput=True)
        dbg_hT = nc.declare_dram_parameter("dbg_hT", [128, G, 128], BF16, isOutput=True)
        dbg_h1 = nc.declare_dram_parameter("dbg_h1", [B, H], F32, isOutput=True)

    xt_full = nc.dram_tensor("xt_full", [N, B], BF16)
    h1_bounce = nc.dram_tensor("h1_bounce", [B, H], F32)
    h1_red = nc.dram_tensor("h1_red", [B, H], F32, addr_space="Shared")

    iota_d = nc.declare_dram_parameter("iotat", [128, CMAX, WIN], BF16, isOutput=False)


    with nc.Block() as _blk:
        @_blk.gpsimd
        def _(gp):
            gp.load_library(library_config.mlp)

    with tile.TileContext(nc) as tc:
        with tc.tile_pool(name="const", bufs=1) as constp, \
             tc.tile_pool(name="sb", bufs=1) as sb, \
             tc.tile_pool(name="gat", bufs=MBUFS) as gat, \
             tc.tile_pool(name="pt", bufs=PBUFS) as ptp, \
             tc.tile_pool(name="ev", bufs=4) as evp, \
             tc.tile_pool(name="ps", bufs=4, space="PSUM") as ps, \
             tc.tile_pool(name="ps2", bufs=2, space="PSUM") as ps2:

            ident = constp.tile([128, 128], F32)
            make_identity(nc, ident[:])

            # ---------- P1: degree ----------
            wdeg_t = sb.tile([128, G, width], F32)
            nc.sync.dma_start(wdeg_t[:], wdeg_d[:])
            deg_t = sb.tile([128, G], F32)
            nc.vector.tensor_reduce(out=deg_t[:], in_=wdeg_t[:],
                                    axis=mybir.AxisListType.X, op=mybir.AluOpType.add)
            sq_t = sb.tile([128, G], F32)
            nc.scalar.activation(out=sq_t[:], in_=deg_t[:],
                                 func=mybir.ActivationFunctionType.Sqrt)
            dinv_t = sb.tile([128, G], F32)
            nc.vector.reciprocal(dinv_t[:], sq_t[:])
            # full-graph deg: every core reduces the full padded layout
            GFULL = N // 128
            degf = sb.tile([128, GFULL], F32)
            GCH = min(16, GFULL)
            for gc in range(0, GFULL, GCH):
                wdf = gat.tile([128, GCH, width], F32, tag="wdegf")
                nc.gpsimd.dma_start(wdf[:], wdegf_d[:, gc:gc + GCH, :])
                nc.vector.tensor_reduce(out=degf[:, gc:gc + GCH], in_=wdf[:],
                                        axis=mybir.AxisListType.X, op=mybir.AluOpType.add)
            sqf = sb.tile([128, GFULL], F32)
            nc.scalar.activation(out=sqf[:], in_=degf[:],
                                 func=mybir.ActivationFunctionType.Sqrt)
            dinvf = sb.tile([128, GFULL], F32)
            nc.vector.reciprocal(dinvf[:], sqf[:])

            # ---------- P2: full x~T build (local, streamed, 8-tile groups) ----------
            XG = min(8, N // 128)
            for tg in [] if "xt" in skip else range(N // (128 * XG)):
                x_t = gat.tile([128, 128 * XG], F32, tag="xin")
                nc.gpsimd.dma_start(x_t[:], x_d[:, tg * 128 * XG:(tg + 1) * 128 * XG])
                xt_g = evp.tile([128, XG, 128], BF16, tag="xtev")
                for j in range(XG):
                    t = tg * XG + j
                    tp = ps.tile([128, 128], F32, space="PSUM", tag="tpose")
                    nc.tensor.transpose(out=tp[:], in_=x_t[:, j * 128:(j + 1) * 128],
                                        identity=ident[:])
                    nc.vector.tensor_scalar_mul(out=xt_g[:, j, :], in0=tp[:],
                                                scalar1=dinvf[:, t:t + 1])
                nc.gpsimd.dma_start(
                    xt_full[tg * 128 * XG:(tg + 1) * 128 * XG, :].rearrange("(g p) b -> p g b", p=128),
                    xt_g[:])

            # ---------- P4: main diffusion ----------
            gidx_t = sb.tile([128, SC * 8], I16)
            nc.sync.dma_start(gidx_t[:], gidx_d[:])
            dstl_t = sb.tile([128, SC], BF16)
            nc.sync.dma_start(dstl_t[:], dstl_d[:])
            wv_t = sb.tile([128, SC], BF16)
            nc.sync.dma_start(wv_t[:], wv_d[:])

            iota_t = constp.tile([128, CMAX, WIN], BF16)
            nc.sync.dma_start(iota_t[:], iota_d[:])

            hT_t = sb.tile([128, G, 128], BF16)   # diffusion output, [dst-k layout]
            nreg_cache = {}
            def nreg(v):
                if v not in nreg_cache:
                    nreg_cache[v] = nc.gpsimd.to_reg(v)
                return nreg_cache[v]
            col = 0
            for w in range(WPC):
                cw = int(CW[w])
                msgs_t = gat.tile([128, CMAX, B], BF16, tag="msgs")
                for s0 in [] if "gather" in skip else range(0, cw, GSP):
                    s1 = min(s0 + GSP, cw)
                    nc.gpsimd.dma_gather(
                        out_ap=msgs_t[:, s0:s1, :], in_ap=xt_full[:],
                        idxs_ap=gidx_t[:, (col + s0) * 8:(col + s1) * 8],
                        num_idxs=(s1 - s0) * 128, num_idxs_reg=nreg((s1 - s0) * 128),
                        elem_size=B, queue_num=(w * 5 + s0 // GSP) % NQ,
                    )
                p_t = ptp.tile([128, CMAX, WIN], BF16, tag="ptile")
                if "pbuild" not in skip:
                  nc.vector.tensor_tensor(
                    out=p_t[:, :cw, :],
                    in0=dstl_t[:, col:col + cw].to_broadcast([128, cw, WIN]),
                    in1=iota_t[:, :cw, :], op=mybir.AluOpType.is_equal)
                  nc.vector.tensor_tensor(
                    out=p_t[:, :cw, :], in0=p_t[:, :cw, :],
                    in1=wv_t[:, col:col + cw].to_broadcast([128, cw, WIN]),
                    op=mybir.AluOpType.mult)
                acc = ps2.tile([64, B], F32, space="PSUM", tag="acc")
                for ch in [] if "mm" in skip else range(cw):
                    nc.tensor.matmul(out=acc[:], lhsT=p_t[:, ch, :],
                                     rhs=msgs_t[:, ch, :],
                                     start=(ch == 0), stop=(ch == cw - 1))
                # eviction: scale by dinv of this window's dsts
                # window w covers dsts [w*64, w*64+64): d = g*128+p
                gsel = (w * WIN) // 128
                poff = (w * WIN) % 128
                nc.vector.tensor_scalar_mul(
                    out=hT_t[poff:poff + WIN, gsel, :], in0=acc[:],
                    scalar1=dinv_t[poff:poff + WIN, gsel:gsel + 1])
                col += cw

            if debug:
                nc.sync.dma_start(dbg_dinv[:], dinv_t[:])
                xtf = sb.tile([128, (N * B) // (128 * B), B], BF16, tag="dbgxt")
                nc.sync.dma_start(xtf[:], xt_full[:].rearrange("(t p) h -> p t h", p=128))
                nc.sync.dma_start(dbg_xt[:].rearrange("(t p) h -> p t h", p=128), xtf[:])
                nc.sync.dma_start(dbg_hT[:], hT_t[:])

            # ---------- P5: MLP ----------
            w1t_t = sb.tile([128, G, H], BF16)
            nc.sync.dma_start(w1t_t[:], w1t_d[:].rearrange("(t p) h -> p t h", p=128))
            h1ps = ps2.tile([128, H], F32, space="PSUM", tag="mlp")
            for t in range(G):
                nc.tensor.matmul(out=h1ps[:], lhsT=hT_t[:, t, :], rhs=w1t_t[:, t, :],
                                 start=(t == 0), stop=(t == G - 1))
            h1sb = sb.tile([128, H], F32)
            nc.vector.tensor_copy(out=h1sb[:], in_=h1ps[:])
            nc.sync.dma_start(h1_bounce[:], h1sb[:])
            if "coll" not in skip:
                nc.gpsimd.collective_compute(
                    "AllReduce", mybir.AluOpType.add,
                    replica_groups=[list(range(NCORES))],
                    ins=[h1_bounce[:]], outs=[h1_red[:]],
                )
            h1r = sb.tile([128, H], F32)
            nc.sync.dma_start(h1r[:], h1_red[:])
            if debug:
                nc.sync.dma_start(dbg_h1[:], h1r[:])
            b1_t = sb.tile([128, H // 128], F32)
            nc.sync.dma_start(b1_t[:], b1_d[:])
            b2_t = sb.tile([128, H // 128], F32)
            nc.sync.dma_start(b2_t[:], b2_d[:])

            # h1T tiles with relu(x+b1): transpose then ACT
            KH = H // 128  # 4
            h1T = sb.tile([128, KH, 128], BF16)
            for t in range(KH):
                tp = ps.tile([128, 128], F32, space="PSUM", tag="tpose")
                nc.tensor.transpose(out=tp[:], in_=h1r[:, t * 128:(t + 1) * 128],
                                    identity=ident[:])
                nc.scalar.activation(out=h1T[:, t, :], in_=tp[:],
                                     func=mybir.ActivationFunctionType.Relu,
                                     bias=b1_t[:, t:t + 1])
            # h2 = relu(h1 @ W2T + b2)
            w2t_t = sb.tile([128, KH, H], BF16)
            nc.sync.dma_start(w2t_t[:], w2t_d[:].rearrange("(t p) h -> p t h", p=128))
            h2ps = ps2.tile([128, H], F32, space="PSUM", tag="mlp")
            for t in range(KH):
                nc.tensor.matmul(out=h2ps[:], lhsT=h1T[:, t, :], rhs=w2t_t[:, t, :],
                                 start=(t == 0), stop=(t == KH - 1))
            h2sb = sb.tile([128, H], F32)
            nc.vector.tensor_copy(out=h2sb[:], in_=h2ps[:])
            h2T = sb.tile([128, KH, 128], BF16)
            for t in range(KH):
                tp = ps.tile([128, 128], F32, space="PSUM", tag="tpose")
                nc.tensor.transpose(out=tp[:], in_=h2sb[:, t * 128:(t + 1) * 128],
                                    identity=ident[:])
                nc.scalar.activation(out=h2T[:, t, :], in_=tp[:],
                                     func=mybir.ActivationFunctionType.Relu,
                                     bias=b2_t[:, t:t + 1])
            # out = h2 @ WfcT + bfc
            wfct_t = sb.tile([128, KH, COUT], BF16)
            nc.sync.dma_start(wfct_t[:], wfct_d[:].rearrange("(t p) h -> p t h", p=128))
            ops_ = ps2.tile([128, COUT], F32, space="PSUM", tag="mlp")
            for t in range(KH):
                nc.tensor.matmul(out=ops_[:], lhsT=h2T[:, t, :], rhs=wfct_t[:, t, :],
                                 start=(t == 0), stop=(t == KH - 1))
            bfc_t = sb.tile([128, COUT], F32)
            nc.sync.dma_start(bfc_t[:], bfc_d[:])
            out_t = sb.tile([128, COUT], F32)
            nc.vector.tensor_add(out=out_t[:], in0=ops_[:], in1=bfc_t[:])
            nc.sync.dma_start(out_d[:], out_t[:])

    return nc


# --------------------------------------------------------------------------
# Public entry point
# --------------------------------------------------------------------------

_BUILD_CACHE = {}


def _get_built(CW, width):
    key = (tuple(int(c) for c in CW), int(width))
    if key not in _BUILD_CACHE:
        nc = build(list(CW), width)
        legalize_waits(nc, max_waits=1)
        mybir.codegen_inst_isa_subclasses(nc)
        _BUILD_CACHE[key] = nc
    return _BUILD_CACHE[key]


def kernel(**inputs) -> np.ndarray:
    in_maps, CW, width = make_inmaps(inputs)
    nc = _get_built(CW, width)
    res = run_bass_kernel_spmd(nc, in_maps, list(range(NCORES)))
    return np.asarray(res.results[0]["out"], np.float32)

